# revision 43
# baseline (speedup 1.0000x reference)
"""Trainium2 Bass kernel for an attention-LSTM caption decoder (v2).

Math notes (verified against the reference):
  - num_pixels == 1 makes softmax attention a no-op: alpha == 1.0, so
    awe = sigmoid(h @ W_beta) * features. W_enc/W_dec/W_full unused.
  - Masked (b, t) rows never re-activate and never influence active rows, so
    h/c freezing is dropped; only output masking matters (lengths sorted
    descending -> active rows at step t are a prefix).
  - All biases in setup_inputs() are zero; detected host-side and compiled out
    (cheap fallbacks kept).

v2 design (vs v1 at ~411us):
  - Column tiling (128x64 mode): every M=64 recurrence matmul is emitted as a
    T0/T1 pair writing PSUM partitions 0-63 / 64-127 concurrently (~1.7x
    measured on HW for N=512 pairs).
  - Folded-H layout: batch lives twice on the partition axis
    (p = b + 64*s, s = H-half), so pointwise ACT/DVE ops run on all 128
    partitions at FD=256/512 instead of 64 partitions at FD=512/1024.
    Gate PSUM = 2 tiles [128,512]: (f,i) -> one sigmoid ACT covers both.
  - E rows ragged-packed to active (t,b) pairs only: E row index == hall row
    index == off[t]+b. ~6 blocks instead of 10.
  - bf16 output staging + DMA (halves write traffic); host converts to f32.
  - Weight DMAs split/ordered by first use.

Distribution (8 cores): recurrence replicated; fc weight + vocab dim sharded
8-way (tensor parallel).
"""

import numpy as np

from concourse import bacc, bass, library_config, mybir, tile
from concourse.bass_utils import run_bass_kernel_spmd

F32 = mybir.dt.float32
BF16 = mybir.dt.bfloat16
I16 = mybir.dt.int16

B = 64
H = 512
T = 20
V = 50257
NCORES = 8
VS = 6284            # per-core vocab shard (8 * 6284 = 50272 >= 50257)
VPAD = NCORES * VS
KC = 4               # k-chunks per 512-wide contraction
GATE_N = 4 * H       # 2048

VCHUNKS = [(i * 512, min(512, VS - i * 512)) for i in range(13)]

# folded gate-column order: psum tile_fi = [f|i], tile_go = [g|o]; T0 gets the
# lo H-half columns, T1 the hi half.  PyTorch gate order i,f,g,o.
_CO = np.r_[512:768, 0:256,          # cb0: f-lo | i-lo   (fi tile, T0)
            768:1024, 256:512,       # cb1: f-hi | i-hi   (fi tile, T1)
            1024:1280, 1536:1792,    # cb2: g-lo | o-lo   (go tile, T0)
            1280:1536, 1792:2048]    # cb3: g-hi | o-hi   (go tile, T1)


def _pack_k(w):
    """[K, N] -> [128, K//128, N] with the contraction dim on partitions."""
    k, n = w.shape
    assert k % 128 == 0
    return np.ascontiguousarray(w.reshape(k // 128, 128, n).transpose(1, 0, 2))


def _pack_idx(a):
    """(n,) int16 -> [128, n//16] replicated into each of the 8 GPSIMD Q7
    cores' 16-partition groups."""
    n = a.shape[0]
    assert n % 16 == 0
    out = np.zeros((128, n // 16), np.int16)
    for c in range(8):
        out[16 * c : 16 * c + 16, :] = a.reshape(n // 16, 16).T
    return out


def _fold(x):
    """[B, H] -> [128, 256] folded: out[b + 64*s, n] = x[b, 256*s + n]."""
    b, h = x.shape
    assert b == 64 and h == 512
    return np.ascontiguousarray(
        x.reshape(64, 2, 256).transpose(1, 0, 2).reshape(128, 256))


def _host_prep(inputs):
    import ml_dtypes

    bf16 = ml_dtypes.bfloat16
    f32 = np.float32
    feats = np.asarray(inputs["features"], f32)
    caps = np.asarray(inputs["captions"]).astype(np.int64)
    lens = np.asarray(inputs["lengths"]).reshape(-1).astype(np.int64)
    table = np.asarray(inputs["embed_table"], f32)

    W_ih = np.asarray(inputs["W_ih"], f32)
    W_hh = np.asarray(inputs["W_hh"], f32)
    b_ih = np.asarray(inputs["b_ih"], f32)
    b_hh = np.asarray(inputs["b_hh"], f32)
    b_beta = np.asarray(inputs["b_beta"], f32)
    b_fc = np.asarray(inputs["b_fc"], f32)
    b_hinit = np.asarray(inputs["b_hinit"], f32)
    b_cinit = np.asarray(inputs["b_cinit"], f32)

    # ragged-batch packing (lengths sorted descending by construction)
    b_t = [int((lens > t).sum()) for t in range(T)]
    off = np.concatenate([[0], np.cumsum(b_t)]).astype(np.int64)
    p_total = int(off[-1])
    t_eff = max(t for t in range(T) if b_t[t] > 0) + 1

    # Step-aligned block packing: greedily pack whole steps into 128-row
    # blocks (padding tails) so each block -- and with it its slice of the
    # vocab projection -- completes at the earliest possible step.  Fall back
    # to dense row packing if padding would add a block.
    mv_min = (p_total + 127) // 128
    cuts = []          # (t_start, t_end_excl) per block
    tcur = 0
    while tcur < t_eff:
        tot, tend = 0, tcur
        while tend < t_eff and tot + b_t[tend] <= 128:
            tot += b_t[tend]
            tend += 1
        cuts.append((tcur, tend))
        tcur = tend
    ro = [0] * (T + 1)
    if len(cuts) <= mv_min:
        mc_ready = []
        fills = []
        for m, (ts, te) in enumerate(cuts):
            pos = 128 * m
            for t in range(ts, te):
                ro[t] = pos
                pos += b_t[t]
            mc_ready.append(te - 1)
            fills.append(pos - 128 * m)
        for t in range(t_eff, T + 1):
            ro[t] = 128 * len(cuts)
        p_pad = 128 * len(cuts)
    else:
        for t in range(T + 1):
            ro[t] = int(off[t])
        p_pad = 128 * mv_min
        mc_ready = []
        for m in range(mv_min):
            need = 128 * (m + 1)
            r = t_eff - 1
            for t in range(t_eff):
                if off[t + 1] >= need:
                    r = t
                    break
            mc_ready.append(r)
        fills = [min(128, p_total - 128 * m) for m in range(mv_min)]
    mv = p_pad // 128

    # E/hall rows: row r = off[t]+b.  Rows 0:64 = features (step 0); rows
    # [64, p_total) = embedding rows, gathered HOST-side (indices are static
    # at build time) and shipped pre-transposed: embT[p, kc, r].
    idx_flat = np.zeros(p_pad, np.int64)
    for t in range(1, T):
        for b in range(b_t[t]):
            idx_flat[ro[t] + b] = caps[b, t - 1]
    emb_rows = table[idx_flat]                   # [p_pad, 512]
    emb_rows[:64] = 0.0
    embT = _pack_k(np.ascontiguousarray(emb_rows.T)).astype(bf16)

    # weights, folded column order
    w2emb = W_ih.T[:H][:, _CO]                       # [512, 2048]
    w2ah = np.vstack([W_ih.T[H:], W_hh.T])[:, _CO]   # [1024, 2048]

    b2 = (b_ih + b_hh)[_CO]
    has_b2 = bool(np.any(b2))
    has_bbeta = bool(np.any(b_beta))
    has_bfc = bool(np.any(b_fc))
    has_binit = bool(np.any(b_hinit)) or bool(np.any(b_cinit))

    # e_load selectors: selA[k, 64t+m] = 1 iff k == ro[t]%128 + m (< 128);
    # selB handles the next-block wrap.
    selA = np.zeros((128, T * 64), f32)
    selB = np.zeros((128, T * 64), f32)
    span = []
    for t in range(T):
        p0 = ro[t] % 128
        mb0 = ro[t] // 128
        sp = (p0 + 64 > 128) and (mb0 + 1 < mv)
        span.append(bool(sp))
        for m in range(64):
            k = p0 + m
            if k < 128:
                selA[k, 64 * t + m] = 1.0
            elif sp:
                selB[k - 128, 64 * t + m] = 1.0

    common = {
        "embT": embT,
        "featT": _pack_k(feats.T.astype(f32)).astype(bf16),
        "w2e": _pack_k(w2emb).astype(bf16),
        "w2ah": _pack_k(w2ah).astype(bf16),
        "wbeta": _pack_k(np.asarray(inputs["W_beta"], f32)).astype(bf16),
        "selA": selA.astype(bf16),
        "selB": selB.astype(bf16),
    }
    # constpack: cols 0:128 ident | 128:384 folded features
    cpk = np.zeros((128, 384), bf16)
    cpk[:, 0:128] = np.eye(128, dtype=f32).astype(bf16)
    cpk[:, 128:384] = _fold(feats).astype(bf16)
    common["constpack"] = cpk
    whc = np.zeros((128, KC, 2 * H), bf16)
    whc[:, :, 0:H] = _pack_k(np.asarray(inputs["W_hinit"], f32)).astype(bf16)
    whc[:, :, H : 2 * H] = _pack_k(np.asarray(inputs["W_cinit"], f32)).astype(bf16)
    common["whcpack"] = whc
    if has_b2:
        common["b2rep"] = np.ascontiguousarray(
            np.tile(b2[None, :], (128, 1)).astype(f32))
    if has_bbeta:
        common["bbeta2"] = np.ascontiguousarray(
            np.tile(_fold(np.tile(b_beta[None, :], (64, 1))), (1, 1))).astype(f32)
    if has_binit:
        # hT layout add: bh[k', nh, 64s+b] = b_hinit[256s+128nh+k']
        bh = np.zeros((128, 2, 128), f32)
        for s in range(2):
            for nh in range(2):
                for kp in range(128):
                    bh[kp, nh, 64 * s : 64 * s + 64] = b_hinit[256 * s + 128 * nh + kp]
        common["bhT"] = bh
        common["bc2"] = _fold(np.tile(b_cinit[None, :], (64, 1))).astype(f32)

    W_fc = np.asarray(inputs["W_fc"], f32)
    wfc_pad = np.zeros((H, VPAD), f32)
    wfc_pad[:, :V] = W_fc
    bfc_pad = np.zeros(VPAD, f32)
    bfc_pad[:V] = b_fc

    in_maps = []
    for k in range(NCORES):
        m = dict(common)
        m["wfc"] = _pack_k(wfc_pad[:, k * VS : (k + 1) * VS]).astype(bf16)
        if has_bfc:
            m["bfcrep"] = np.ascontiguousarray(
                np.tile(bfc_pad[k * VS : (k + 1) * VS][None, :], (128, 1))
            ).astype(f32)
        in_maps.append(m)

    meta = {
        "b_t": b_t, "ro": ro, "p_total": p_total,
        "p_pad": p_pad, "t_eff": t_eff, "span": span,
        "mc_ready": mc_ready, "fills": fills,
        "has_b2": has_b2, "has_bbeta": has_bbeta, "has_bfc": has_bfc,
        "has_binit": has_binit,
    }
    return in_maps, meta


def build_program(meta):
    b_t = meta["b_t"]
    ro = meta["ro"]
    p_pad = meta["p_pad"]
    t_eff = meta["t_eff"]
    span = meta["span"]
    mc_ready = meta["mc_ready"]
    fills = meta["fills"]
    mv = p_pad // 128
    has_b2 = meta["has_b2"]
    has_bbeta = meta["has_bbeta"]
    has_bfc = meta["has_bfc"]
    has_binit = meta["has_binit"]

    nc = bacc.Bacc(num_swdge_queues=1)

    embT_d = nc.declare_dram_parameter("embT", [128, KC, p_pad], BF16, isOutput=False)
    featT_d = nc.declare_dram_parameter("featT", [128, KC, B], BF16, isOutput=False)
    w2e_d = nc.declare_dram_parameter("w2e", [128, KC, GATE_N], BF16, isOutput=False)
    w2ah_d = nc.declare_dram_parameter("w2ah", [128, 8, GATE_N], BF16, isOutput=False)
    wbeta_d = nc.declare_dram_parameter("wbeta", [128, KC, H], BF16, isOutput=False)
    constpack_d = nc.declare_dram_parameter("constpack", [128, 384], BF16, isOutput=False)
    whcpack_d = nc.declare_dram_parameter("whcpack", [128, KC, 2 * H], BF16, isOutput=False)
    selA_d = nc.declare_dram_parameter("selA", [128, T * 64], BF16, isOutput=False)
    selB_d = nc.declare_dram_parameter("selB", [128, T * 64], BF16, isOutput=False)
    wfc_d = nc.declare_dram_parameter("wfc", [128, KC, VS], BF16, isOutput=False)
    if has_b2:
        b2rep_d = nc.declare_dram_parameter("b2rep", [128, GATE_N], F32, isOutput=False)
    if has_bbeta:
        bbeta2_d = nc.declare_dram_parameter("bbeta2", [128, 256], F32, isOutput=False)
    if has_binit:
        bhT_d = nc.declare_dram_parameter("bhT", [128, 2, 128], F32, isOutput=False)
        bc2_d = nc.declare_dram_parameter("bc2", [128, 256], F32, isOutput=False)
    if has_bfc:
        bfcrep_d = nc.declare_dram_parameter("bfcrep", [128, VS], F32, isOutput=False)
    out_d = nc.declare_dram_parameter("out", [p_pad, VS], BF16, isOutput=True)

    def mm(out, lhsT, rhs, start, stop):
        nc.tensor.matmul(out, lhsT, rhs, start=start, stop=stop)

    # vocab item (mc, vo, vw) runnable once all hall rows of block mc are
    # written, i.e. after step mc_ready[mc]'s h transposes.
    vqueue = [(mc, vo, vw) for mc in range(mv) for (vo, vw) in VCHUNKS]

    # per-step hall write segments: (mc, local_lo, src_lo, n)
    hall_segs = []
    for t in range(T):
        segs = []
        lo, n = ro[t], b_t[t]
        while n > 0:
            mc = lo // 128
            ll = lo % 128
            take = min(128 - ll, n)
            segs.append((mc, ll, lo - ro[t], take))
            lo += take
            n -= take
        hall_segs.append(segs)

    # E-block needed latest by step t (e_load reads 64 rows from ro[t]):
    # block ro[t]//128, plus next block when spanning.
    eb_need = [ro[t] // 128 + (1 if span[t] else 0) for t in range(t_eff)]

    SIG = mybir.ActivationFunctionType.Sigmoid
    TANH = mybir.ActivationFunctionType.Tanh

    with tile.TileContext(nc) as tc:
        with (
            tc.tile_pool(name="const", bufs=1) as constp,
            tc.tile_pool(name="res", bufs=1) as resp,
            tc.tile_pool(name="state", bufs=1) as statep,
            tc.tile_pool(name="step", bufs=1) as sp,
            tc.tile_pool(name="stage", bufs=3) as stp,
            tc.tile_pool(name="gates", bufs=2, space="PSUM") as gps,
            tc.tile_pool(name="tpp", bufs=1, space="PSUM") as tpp,
            tc.tile_pool(name="betap", bufs=1, space="PSUM") as bpp,
            tc.tile_pool(name="fill", bufs=2, space="PSUM") as fillp,
        ):
            # --- small constants first (featT leads: h0/c0 + block-0 E) ---
            featT = constp.tile([128, KC, B], BF16)
            nc.sync.dma_start(featT[:], featT_d[:])
            whc = constp.tile([128, KC, 2 * H], BF16, name="whc")
            nc.sync.dma_start(whc[:, :, 0:H], whcpack_d[:, :, 0:H])
            nc.sync.dma_start(whc[:, :, H : 2 * H], whcpack_d[:, :, H : 2 * H])
            cp = constp.tile([128, 384], BF16, name="cp")
            nc.sync.dma_start(cp[:], constpack_d[:])
            selA = constp.tile([128, T * 64], BF16, name="selA")
            nc.sync.dma_start(selA[:], selA_d[:])
            selB = constp.tile([128, T * 64], BF16, name="selB")
            if any(span):
                nc.sync.dma_start(selB[:], selB_d[:])
            ident = cp[:, 0:128]
            feat2 = cp[:, 128:384]
            if has_bbeta:
                bbeta2 = constp.tile([128, 256], F32, name="bbeta2")
                nc.sync.dma_start(bbeta2[:], bbeta2_d[:])
            if has_binit:
                bhT = constp.tile([128, 2, 128], F32, name="bhT")
                nc.sync.dma_start(bhT[:], bhT_d[:])
                bc2 = constp.tile([128, 256], F32, name="bc2")
                nc.sync.dma_start(bc2[:], bc2_d[:])

            # --- persistent residents ---
            w2ah = resp.tile([128, 8, GATE_N], BF16)
            wbeta = resp.tile([128, KC, H], BF16)
            e_sb = resp.tile([128, mv, 4, 512], BF16, name="E")
            hall_t = [
                resp.tile([128, KC, 128], BF16, tag=f"hall{mc}", name=f"hall{mc}")
                for mc in range(mv)
            ]
            for m2 in range(mv):
                if fills[m2] < 128:
                    nc.vector.memset(hall_t[m2][:, :, fills[m2] :], 0.0)
            if has_b2:
                b2rep = resp.tile([128, GATE_N], F32)
            if has_bfc:
                bfcrep = resp.tile([128, VS], F32)
            wfc = resp.tile([128, KC, VS], BF16)

            hT = statep.tile([128, 2, 128], BF16)
            aweT = statep.tile([128, 2, 128], BF16)
            c2 = statep.tile([128, 256], F32)
            h2 = statep.tile([128, 256], BF16)

            # ============ helpers ============
            vstate = {"q": 0, "alt": 0}

            def vocab_item():
                mc, vo, vw = vqueue[vstate["q"]]
                vstate["q"] += 1
                pv = fillp.tile([128, 512], F32, tag="fill")
                for kc in range(KC):
                    mm(pv[:, :vw], hall_t[mc][:, kc, :], wfc[:, kc, vo : vo + vw],
                       start=(kc == 0), stop=(kc == KC - 1))
                st = stp.tile([128, 512], BF16, tag="st")
                if has_bfc:
                    nc.vector.tensor_add(st[:, :vw], pv[:, :vw],
                                         bfcrep[:, vo : vo + vw])
                elif vstate["alt"] == 0:
                    nc.scalar.copy(st[:, :vw], pv[:, :vw])
                else:
                    nc.vector.tensor_copy(st[:, :vw], pv[:, :vw])
                vstate["alt"] ^= 1
                nc.sync.dma_start(
                    out_d[mc * 128 : (mc + 1) * 128, vo : vo + vw], st[:, :vw])

            def emit_vocab(t, cap):
                n = 0
                while (vstate["q"] < len(vqueue) and n < cap
                       and mc_ready[vqueue[vstate["q"]][0]] < t):
                    if t < t_eff - 2:
                        # hold a few eligible items back so the final steps'
                        # pointwise windows still have PE work to chew on
                        eligible = sum(
                            1 for i in range(vstate["q"], len(vqueue))
                            if mc_ready[vqueue[i][0]] < t)
                        if eligible <= 5:
                            break
                    vocab_item()
                    n += 1

            def hT_slice(src, kc):
                # contraction chunk kc -> hT/aweT [:, kc%2, 64*(kc//2):+64]
                s, nh = kc // 2, kc % 2
                return src[:, nh, 64 * s : 64 * s + 64]

            def e_load(t, gfi, ggo):
                sA = selA[:, 64 * t : 64 * t + 64]
                for tile_, cbl in ((gfi, 0), (ggo, 2)):
                    mb = ro[t] // 128
                    mm(tile_[0:64, :], sA, e_sb[:, mb, cbl, :],
                       start=True, stop=False)
                    mm(tile_[64:128, :], sA, e_sb[:, mb, cbl + 1, :],
                       start=True, stop=False)
                    if span[t]:
                        sB = selB[:, 64 * t : 64 * t + 64]
                        mm(tile_[0:64, :], sB, e_sb[:, mb + 1, cbl, :],
                           start=False, stop=False)
                        mm(tile_[64:128, :], sB, e_sb[:, mb + 1, cbl + 1, :],
                           start=False, stop=False)

            def h_trans(t, hsrc):
                """Transpose folded h (prev step's output) into hT + hall."""
                tph = tpp.tile([128, 256], BF16, tag="tp")
                nc.tensor.transpose(tph[:, 0:128], hsrc[:, 0:128], ident)
                nc.tensor.transpose(tph[:, 128:256], hsrc[:, 128:256], ident)
                for nh in range(2):
                    nc.vector.tensor_copy(hT[:, nh, :],
                                          tph[:, 128 * nh : 128 * nh + 128])
                for kc in range(KC):
                    s, nh = kc // 2, kc % 2
                    for (mc, ll, sl, seg_n) in hall_segs[t]:
                        nc.vector.tensor_copy(
                            hall_t[mc][:, kc, ll : ll + seg_n],
                            tph[:, 128 * nh + 64 * s + sl :
                                   128 * nh + 64 * s + sl + seg_n])

            def beta_mms():
                bp = bpp.tile([128, 256], F32, tag="beta")
                for kc in range(KC):
                    lhs = hT_slice(hT, kc)
                    mm(bp[0:64, :], lhs, wbeta[:, kc, 0:256],
                       start=(kc == 0), stop=(kc == KC - 1))
                    mm(bp[64:128, :], lhs, wbeta[:, kc, 256:512],
                       start=(kc == 0), stop=(kc == KC - 1))
                if has_bbeta:
                    nc.vector.tensor_add(bp[:], bp[:], bbeta2[:])
                return bp

            def h_part(gfi, ggo):
                for tile_, cbl in ((gfi, 0), (ggo, 2)):
                    for kc in range(KC):
                        lhs = hT_slice(hT, kc)
                        mm(tile_[0:64, :], lhs, w2ah[:, 4 + kc, cbl * 512 :
                                                     cbl * 512 + 512],
                           start=False, stop=False)
                        mm(tile_[64:128, :], lhs, w2ah[:, 4 + kc, (cbl + 1) * 512 :
                                                       (cbl + 1) * 512 + 512],
                           start=False, stop=False)

            def sig_awe(bp):
                sigb2 = sp.tile([128, 256], BF16, tag="sigb")
                nc.scalar.activation(sigb2[:], bp[:], SIG)
                awe2 = sp.tile([128, 256], BF16, tag="awe2")
                nc.vector.tensor_mul(awe2[:], sigb2[:], feat2)
                tpa = tpp.tile([128, 256], BF16, tag="tp")
                nc.tensor.transpose(tpa[:, 0:128], awe2[:, 0:128], ident)
                nc.tensor.transpose(tpa[:, 128:256], awe2[:, 128:256], ident)
                for nh in range(2):
                    nc.vector.tensor_copy(aweT[:, nh, :],
                                          tpa[:, 128 * nh : 128 * nh + 128])

            def awe_mms(gfi, ggo):
                # fi tile first so sig_fi starts while go streams
                for tile_, cbl in ((gfi, 0), (ggo, 2)):
                    for kc in range(KC):
                        lhs = hT_slice(aweT, kc)
                        mm(tile_[0:64, :], lhs, w2ah[:, kc, cbl * 512 :
                                                     cbl * 512 + 512],
                           start=False, stop=(kc == KC - 1))
                        mm(tile_[64:128, :], lhs, w2ah[:, kc, (cbl + 1) * 512 :
                                                       (cbl + 1) * 512 + 512],
                           start=False, stop=(kc == KC - 1))

            def pointwise(gfi, ggo):
                sigfi = sp.tile([128, 512], F32, tag="sigfi")
                nc.scalar.activation(sigfi[:], gfi[:], SIG)
                nc.vector.tensor_mul(c2[:], c2[:], sigfi[:, 0:256])
                tanhg = sp.tile([128, 256], F32, tag="tanhg")
                nc.scalar.activation(tanhg[:], ggo[:, 0:256], TANH)
                t2 = sp.tile([128, 256], F32, tag="t2")
                nc.vector.tensor_mul(t2[:], sigfi[:, 256:512], tanhg[:])
                nc.vector.tensor_add(c2[:], c2[:], t2[:])
                sigo = sp.tile([128, 256], BF16, tag="sigo")
                nc.scalar.activation(sigo[:], ggo[:, 256:512], SIG)
                tanhc = sp.tile([128, 256], BF16, tag="tanhc")
                nc.scalar.activation(tanhc[:], c2[:], TANH)
                nc.vector.tensor_mul(h2[:], sigo[:], tanhc[:])

            # ============ prep phase ============
            with (
                tc.tile_pool(name="prew", bufs=1) as prew,
            ):
                # sync ring (parallel with scalar ring): embT first so the PE
                # can chew through E-prep during the weight-stream head, then
                # the awe half of w2ah.
                embT = prew.tile([128, KC, p_pad], BF16, name="embT")
                nc.sync.dma_start(embT[:], embT_d[:])
                nc.sync.dma_start(w2ah[:, 0:4, :], w2ah_d[:, 0:4, :])

                # warm the sigmoid/tanh ACT table during the DMA wait
                warm = sp.tile([1, 2], F32, tag="warm")
                nc.scalar.activation(warm[:], cp[0:1, 0:2], SIG)

                # scalar ring: remaining weights in first-use order
                nc.scalar.dma_start(wbeta[:], wbeta_d[:])
                w2e = prew.tile([128, KC, GATE_N], BF16, bufs=1)
                nc.scalar.dma_start(w2e[:], w2e_d[:])
                nc.scalar.dma_start(w2ah[:, 4:8, :], w2ah_d[:, 4:8, :])
                if has_b2:
                    nc.scalar.dma_start(b2rep[:], b2rep_d[:])
                if has_bfc:
                    nc.scalar.dma_start(bfcrep[:], bfcrep_d[:])
                nc.scalar.dma_start(wfc[:, :, 0:3072], wfc_d[:, :, 0:3072])
                nc.scalar.dma_start(wfc[:, :, 3072:VS], wfc_d[:, :, 3072:VS])

                # h0 (transposed directly into hT) and c0 (folded)
                for jb in range(KC):
                    hps = fillp.tile([128, 512], F32, tag="fill")
                    for kc in range(KC):
                        mm(hps[:, 0:B], whc[:, kc, jb * 128 : (jb + 1) * 128],
                           featT[:, kc, :], start=(kc == 0), stop=(kc == KC - 1))
                    s, nh = jb // 2, jb % 2
                    dst = hT[:, nh, 64 * s : 64 * s + 64]
                    if has_binit:
                        nc.vector.tensor_add(dst, hps[:, 0:B],
                                             bhT[:, nh, 64 * s : 64 * s + 64])
                    else:
                        nc.vector.tensor_copy(dst, hps[:, 0:B])
                cps = fillp.tile([128, 512], F32, tag="fill")
                for kc in range(KC):
                    lhs = featT[:, kc, :]
                    mm(cps[0:64, 0:256], lhs, whc[:, kc, H : H + 256],
                       start=(kc == 0), stop=(kc == KC - 1))
                    mm(cps[64:128, 0:256], lhs, whc[:, kc, H + 256 : H + 512],
                       start=(kc == 0), stop=(kc == KC - 1))
                if has_binit:
                    nc.vector.tensor_add(c2[:], cps[:, 0:256], bc2[:])
                else:
                    nc.vector.tensor_copy(c2[:], cps[:, 0:256])

                # ---- E-prep ----
                ep_alt = [0]

                def e_copy(dst, src, cb):
                    if has_b2:
                        # b2 folded into E so e_load carries the bias
                        nc.vector.tensor_add(dst, src,
                                             b2rep[:, cb * 512 : (cb + 1) * 512])
                    elif ep_alt[0] == 0:
                        nc.scalar.copy(dst, src)
                    else:
                        nc.vector.tensor_copy(dst, src)
                    ep_alt[0] ^= 1

                def e_pre0():
                    """Block 0: rows 0:64 features (T0), 64:128 emb (T1)."""
                    for cb in range(4):
                        pse = fillp.tile([128, 512], F32, tag="fill")
                        for kc in range(KC):
                            mm(pse[0:64, :], featT[:, kc, :],
                               w2e[:, kc, cb * 512 : (cb + 1) * 512],
                               start=(kc == 0), stop=(kc == KC - 1))
                            mm(pse[64:128, :], embT[:, kc, 64:128],
                               w2e[:, kc, cb * 512 : (cb + 1) * 512],
                               start=(kc == 0), stop=(kc == KC - 1))
                        e_copy(e_sb[:, 0, cb, :], pse[:], cb)

                def e_pre(mb):
                    """Full embedding block mb (rows 128mb..128mb+128)."""
                    for cb in range(4):
                        pse = fillp.tile([128, 512], F32, tag="fill")
                        for kc in range(KC):
                            mm(pse[:, :], embT[:, kc, 128 * mb : 128 * mb + 128],
                               w2e[:, kc, cb * 512 : (cb + 1) * 512],
                               start=(kc == 0), stop=(kc == KC - 1))
                        e_copy(e_sb[:, mb, cb, :], pse[:], cb)

                # ---- steps ----
                def step(t):
                    gfi = gps.tile([128, 512], F32, tag="gfi", name="gfi")
                    ggo = gps.tile([128, 512], F32, tag="ggo", name="ggo")
                    e_load(t, gfi, ggo)
                    if t > 0:
                        h_trans(t - 1, h2)
                    bp = beta_mms()
                    h_part(gfi, ggo)
                    sig_awe(bp)
                    awe_mms(gfi, ggo)
                    pointwise(gfi, ggo)
                    emit_vocab(t, 4 if t < t_eff - 6 else 6)

                # interleave: block-0 E, then steps, with e_pre(mb) emitted
                # one step before the first step that e_loads it.
                e_pre0()
                state_mb = {"next": 1, "tgt": 0}
                for t in range(t_eff):
                    state_mb["tgt"] = max(state_mb["tgt"],
                                          eb_need[min(t + 3, t_eff - 1)])
                    while state_mb["next"] <= min(state_mb["tgt"], mv - 1):
                        e_pre(state_mb["next"])
                        state_mb["next"] += 1
                    step(t)
                # remaining blocks (pad rows, never read by e_load) skipped

            h_trans(t_eff - 1, h2)
            while vstate["q"] < len(vqueue):
                vocab_item()

    nc.finalize()
    return nc


def kernel(**inputs):
    in_maps, meta = _host_prep(inputs)
    nc = build_program(meta)
    res = run_bass_kernel_spmd(nc, in_maps, core_ids=list(range(NCORES)))
    results = res.results

    b_t = meta["b_t"]
    ro = meta["ro"]
    full = np.zeros((B, T, VPAD), np.float32)
    for k in range(NCORES):
        o = np.asarray(results[k]["out"]).astype(np.float32)
        for t in range(T):
            bt = b_t[t]
            if bt:
                full[:bt, t, k * VS : (k + 1) * VS] = o[ro[t] : ro[t] + bt]
    return full[:, :, :V]
